# revision 5
# baseline (speedup 1.0000x reference)
"""Multi-head causal attention (B=4, S=2048, E=1024, H=16, D=64) on 8 TRN2
NeuronCores. Head-sharded tensor parallelism: each core computes 2 heads for
all batches plus its 128-row slice of the output projection; the host sums
the 8 partial outputs.

Self-contained: hardcodes shapes/sharding; only depends on /opt/trn_rl_repo.
"""
import sys
from contextlib import ExitStack

sys.path.insert(0, "/opt/trn_rl_repo")

import numpy as np
import ml_dtypes

import concourse.bass as bass  # noqa: F401  (registers engine types)
import concourse.bacc as bacc
import concourse.mybir as mybir
import concourse.tile as tile
from concourse.bass_utils import run_bass_kernel_spmd
from concourse.masks import make_upper_triangular

BF16 = mybir.dt.bfloat16
F32 = mybir.dt.float32
NBF = ml_dtypes.bfloat16

B, S, E, H, D = 4, 2048, 1024, 16, 64
NCORES = 8
HPC = 2          # heads per core
D2 = HPC * D     # 128
QT_ = 512        # q tile width
KC_ = 128        # k chunk width
EXP_FN = mybir.ActivationFunctionType.Exp
MULT = mybir.AluOpType.mult


def build_program(b=B, s=S, e=E, repeat=1):
    """One SPMD program, identical on all 8 cores."""
    assert s % QT_ == 0 and e % 128 == 0
    EC = e // 128            # contraction chunks
    NQ = s // QT_            # q tiles per batch
    NK = s // KC_            # k chunks per batch
    ST = s // 128            # s chunks per batch

    nc = bacc.Bacc("TRN2", target_bir_lowering=False, debug=False,
                   num_devices=NCORES)
    with tile.TileContext(nc) as tc, ExitStack() as ctx:
        with tc.tile_pool(name="dram", bufs=1, space="DRAM") as dram:
            xb_d = dram.tile([b * s, e], BF16, kind="ExternalInput",
                             name="xb", uniquify=False)
            wq_d = dram.tile([e, D2], BF16, kind="ExternalInput",
                             name="wq", uniquify=False)
            wk_d = dram.tile([e, D2], BF16, kind="ExternalInput",
                             name="wk", uniquify=False)
            wv_d = dram.tile([e, D2], BF16, kind="ExternalInput",
                             name="wv", uniquify=False)
            wo_d = dram.tile([D2, e], BF16, kind="ExternalInput",
                             name="wo", uniquify=False)
            bq_d = dram.tile([D2, 1], F32, kind="ExternalInput",
                             name="bq", uniquify=False)
            bk_d = dram.tile([D2, 1], F32, kind="ExternalInput",
                             name="bk", uniquify=False)
            out_d = dram.tile([b * s, e], F32, kind="ExternalOutput",
                              name="out", uniquify=False)

            const = ctx.enter_context(tc.tile_pool(name="const", bufs=1))
            wpool = ctx.enter_context(tc.tile_pool(name="wpool", bufs=1))
            xtp = ctx.enter_context(tc.tile_pool(name="xtp", bufs=2))
            qkp = ctx.enter_context(tc.tile_pool(name="qkp", bufs=2))
            vp = ctx.enter_context(tc.tile_pool(name="vp", bufs=2))
            etp = ctx.enter_context(tc.tile_pool(name="etp", bufs=6))
            rp = ctx.enter_context(tc.tile_pool(name="rp", bufs=2))
            orp = ctx.enter_context(tc.tile_pool(name="orp", bufs=2))
            pp = ctx.enter_context(tc.tile_pool(name="pp", bufs=1, space="PSUM"))

            # constants
            tri = const.tile([128, 128], BF16)
            make_upper_triangular(nc, tri[:], val=1.0, diag=True)
            ones64 = const.tile([1, 64], F32)
            nc.vector.memset(ones64[:], 1.0)
            bq_sb = const.tile([D2, 1], F32)
            nc.sync.dma_start(out=bq_sb[:], in_=bq_d[:])
            bk_sb = const.tile([D2, 1], F32)
            nc.sync.dma_start(out=bk_sb[:], in_=bk_d[:])

            # weights, chunked along contraction dim
            wq_sb = wpool.tile([128, EC, D2], BF16)
            nc.sync.dma_start(out=wq_sb[:], in_=wq_d.rearrange("(c p) d -> p c d", p=128))
            wk_sb = wpool.tile([128, EC, D2], BF16)
            nc.sync.dma_start(out=wk_sb[:], in_=wk_d.rearrange("(c p) d -> p c d", p=128))
            wv_sb = wpool.tile([128, EC, D2], BF16)
            nc.sync.dma_start(out=wv_sb[:], in_=wv_d.rearrange("(c p) d -> p c d", p=128))
            wo_sb = wpool.tile([D2, e], BF16)
            nc.sync.dma_start(out=wo_sb[:], in_=wo_d[:])

            def body(_iv=None):
                for bi in range(b):
                    row0 = bi * s
                    # ---- transposed x into SBUF: xt[:, ec, :] = x[b].T chunk
                    xt = xtp.tile([128, EC, s], BF16, name="xt")
                    for ec in range(EC):
                        nc.sync.dma_start(
                            out=xt[:, ec, :],
                            in_=xb_d[row0:row0 + s, ec * 128:(ec + 1) * 128],
                            transpose=True)

                    # ---- QT / KT projections: [D2, s]
                    qt = qkp.tile([D2, s], BF16, name="qt")
                    kt = qkp.tile([D2, s], BF16, name="kt")
                    for st in range(s // 512):
                        cs = slice(st * 512, (st + 1) * 512)
                        psq = pp.tile([128, 512], F32, name="psq", tag="proj", bufs=2)
                        for ec in range(EC):
                            nc.tensor.matmul(psq[:], wq_sb[:, ec, :], xt[:, ec, cs],
                                             start=(ec == 0), stop=(ec == EC - 1))
                        nc.vector.tensor_scalar_add(qt[:, cs], psq[:], bq_sb[:])
                        psk = pp.tile([128, 512], F32, name="psk", tag="proj", bufs=2)
                        for ec in range(EC):
                            nc.tensor.matmul(psk[:], wk_sb[:, ec, :], xt[:, ec, cs],
                                             start=(ec == 0), stop=(ec == EC - 1))
                        nc.vector.tensor_scalar_add(kt[:, cs], psk[:], bk_sb[:])

                    # ---- V natural layout + ones column: per head [128, NK, 65]
                    v0 = vp.tile([128, NK, 65], BF16, name="v0")
                    v1 = vp.tile([128, NK, 65], BF16, name="v1")
                    for sc in range(ST):
                        psv = pp.tile([128, 128], F32, name="psv", tag="proj", bufs=2)
                        for ec in range(EC):
                            nc.tensor.matmul(psv[:], xt[:, ec, sc * 128:(sc + 1) * 128],
                                             wv_sb[:, ec, :],
                                             start=(ec == 0), stop=(ec == EC - 1))
                        for h, vt in enumerate((v0, v1)):
                            nc.vector.tensor_copy(vt[:, sc, 0:64],
                                                  psv[:, h * 64:(h + 1) * 64])
                            nc.gpsimd.memset(vt[:, sc, 64:65], 1.0)

                    # ---- attention + normalized transposed output [D2, s]
                    ot = qkp.tile([D2, s], BF16, name="ot")
                    for qi in range(NQ):
                        pso = [pp.tile([65, 512], F32, name=f"pso{h}",
                                       tag=f"pso{h}", bufs=1) for h in range(HPC)]
                        nkc = (qi + 1) * (QT_ // KC_)
                        for kc in range(nkc):
                            dj = kc - qi * (QT_ // KC_)
                            qoff = KC_ * dj if dj >= 0 else 0
                            n = 512 - qoff
                            for h in range(HPC):
                                hs = slice(h * 64, (h + 1) * 64)
                                vt = (v0, v1)[h]
                                pss = pp.tile([128, 512], F32, name=f"pss{h}",
                                              tag="pss", bufs=2)
                                nc.tensor.matmul(
                                    pss[:, 0:n],
                                    kt[hs, kc * 128:(kc + 1) * 128],
                                    qt[hs, qi * 512 + qoff:(qi + 1) * 512],
                                    start=True, stop=True)
                                et = etp.tile([128, 512], BF16, name="et")
                                nc.scalar.activation(et[:, 0:n], pss[:, 0:n],
                                                     EXP_FN, scale=0.125)
                                if dj >= 0:
                                    nc.vector.tensor_tensor(
                                        et[:, 0:128], et[:, 0:128], tri[:], MULT)
                                nc.tensor.matmul(
                                    pso[h][:, qoff:512], vt[:, kc, :], et[:, 0:n],
                                    start=(kc == 0), stop=(kc == nkc - 1),
                                    skip_group_check=True)
                        for h in range(HPC):
                            r1 = rp.tile([1, 512], F32, name="r1")
                            nc.vector.reciprocal(r1[:], pso[h][64:65, :])
                            psr = pp.tile([64, 512], F32, name="psr",
                                          tag="psr", bufs=1)
                            nc.tensor.matmul(psr[:], ones64[:], r1[:],
                                             start=True, stop=True)
                            rb = rp.tile([64, 512], F32, name="rb")
                            nc.scalar.copy(rb[:], psr[:])
                            nc.vector.tensor_tensor(
                                ot[h * 64:(h + 1) * 64, qi * 512:(qi + 1) * 512],
                                pso[h][0:64, :], rb[:], MULT)

                    # ---- output projection partial: out[s,:] = ot.T @ wo
                    for sc in range(ST):
                        orow = orp.tile([128, e], F32, name="orow")
                        for eh in range(e // 512):
                            psf = pp.tile([128, 512], F32, name="psf",
                                          tag="psf", bufs=1)
                            nc.tensor.matmul(psf[:], ot[:, sc * 128:(sc + 1) * 128],
                                             wo_sb[:, eh * 512:(eh + 1) * 512],
                                             start=True, stop=True)
                            nc.any.tensor_copy(orow[:, eh * 512:(eh + 1) * 512],
                                               psf[:])
                        nc.sync.dma_start(
                            out=out_d[row0 + sc * 128:row0 + (sc + 1) * 128, :],
                            in_=orow[:])

            if repeat == 1:
                body()
            else:
                with tc.For_i(0, repeat, 1) as iv:
                    body(iv)

    nc.compile()
    return nc


_PROG = None


def _prep_in_maps(x, Wq, Wk, Wv, Wo, bq, bk):
    x = np.asarray(x, np.float32)
    b, s, e = x.shape
    xb = np.ascontiguousarray(x.reshape(b * s, e)).astype(NBF)
    maps = []
    for c in range(NCORES):
        h0 = c * HPC
        def wcat(W):
            W = np.asarray(W, np.float32)
            return np.ascontiguousarray(
                np.concatenate([W[h0 + i] for i in range(HPC)], axis=1)
            ).astype(NBF)
        def bcat(bv_):
            bv_ = np.asarray(bv_, np.float32)
            return np.ascontiguousarray(
                np.concatenate([bv_[h0 + i] for i in range(HPC)])
            ).reshape(D2, 1).astype(np.float32)
        wo_sl = np.ascontiguousarray(
            np.asarray(Wo, np.float32)[h0 * D:(h0 + HPC) * D, :]).astype(NBF)
        maps.append({
            "xb": xb, "wq": wcat(Wq), "wk": wcat(Wk), "wv": wcat(Wv),
            "wo": wo_sl, "bq": bcat(bq), "bk": bcat(bk),
        })
    return maps


def kernel(x, Wq, bq, Wk, bk, Wv, bv, Wo, bo):
    global _PROG
    if _PROG is None:
        _PROG = build_program()
    x = np.asarray(x, np.float32)
    Wo = np.asarray(Wo, np.float32)
    maps = _prep_in_maps(x, Wq, Wk, Wv, Wo, bq, bk)
    res = run_bass_kernel_spmd(_PROG, maps, core_ids=list(range(NCORES)))
    acc = res.results[0]["out"].astype(np.float32)
    for c in range(1, NCORES):
        acc = acc + res.results[c]["out"]
    bias_vec = (np.asarray(bv, np.float32).reshape(-1) @ Wo
                + np.asarray(bo, np.float32))
    acc = acc + bias_vec[None, :]
    return acc.reshape(B, S, E)
